# revision 28
# baseline (speedup 1.0000x reference)
"""Trainium2 Bass kernel for nn_CausalRankKAttention.

Blend of banded-softmax attention and cumsum linear attention, per (n,h) pair.
16 pairs sharded over 8 NeuronCores (2 pairs/core), no cross-core comm.

Design (v6):
  - feature map phi(x)=tanh(x)+1 on HOST; only the exp table ever loads on ACT.
  - all matmuls bf16 (PE streams ~1 col/cycle at 1.2GHz regardless of dtype;
    fp8 DoubleRow measured no faster).
  - per block lb (ascending, = linear chunk lb):
      tail: forward scores q-block lb vs s-blocks [0, lb) -> wide psum
        [128, lb*128], one ACT exp with fused accum_out -> denominator tail.
      band: transposed tile st[k=lb, q in lb..lb+1] -> exp -> mask.
      linear: transposed scores -> causal mask -> kn state delta.
  - key trick: with a binary key mask, vsm == vlin == [v, klm]; the band "mv"
    matmul and the linear "atv" matmul share their stationary tensor, so one
    384-col matmul computes both (rhs = [st_m | at] written side by side by
    DVE), with the inter matmul accumulating into the last 128 cols of the
    same psum group. 5 small matmuls per iteration instead of 7.
  - dependent matmuls trail one iteration; band+linear share one [128, 1024]
    psum tile per iteration, drained by a single [65, 384] CAST.
  - outputs are RAW numerators/denominators; normalize + blend on host.
"""

import numpy as np
import ml_dtypes

import concourse.bass as bass
import concourse.bacc as bacc
import concourse.mybir as mybir
import concourse.tile as tile
from concourse import bass_utils

F32 = mybir.dt.float32
BF16 = mybir.dt.bfloat16
AF = mybir.ActivationFunctionType
OP = mybir.AluOpType

N, L, H, E = 2, 2048, 8, 64
NB = L // 128            # 16 blocks/chunks of 128
TEMP = float(1.0 / np.sqrt(E))
EPS = 1e-6
PAIRS_PER_CORE = 2
NCORES = 8

_cached = {}


def build_nc():
    nc = bacc.Bacc("TRN2", target_bir_lowering=False, debug=False,
                   num_devices=NCORES)
    P = PAIRS_PER_CORE
    # ---- dram tensors (per core) ----
    # qkt[p, :, 0] = kt (k^T + gate ext row), [p, :, 1] = qt (q^T + ones row)
    qkt = nc.dram_tensor("qkt", [P, 65, 2, L], BF16, kind="ExternalInput")
    sg = nc.dram_tensor("sg", [P, 64, 2, L], BF16, kind="ExternalInput")
    # vvkn: [vv (NB*65) | sgkn (NB*64)]
    vvkn = nc.dram_tensor("vvkn", [P, 128, NB * 65 + NB * 64], BF16,
                          kind="ExternalInput")
    m01d = nc.dram_tensor("m01d", [128, 256], BF16, kind="ExternalInput")
    svlv = nc.dram_tensor("svlv", [P, 65, NB, 384], BF16, kind="ExternalOutput")
    # two accumulator columns per block (tail split in halves); host sums
    tails = nc.dram_tensor("tails", [P, 128, 2 * NB], F32, kind="ExternalOutput")

    with tile.TileContext(nc) as tc:
        with (
            tc.tile_pool(name="const", bufs=1) as constp,
            tc.tile_pool(name="io", bufs=2) as iop,
            tc.tile_pool(name="acc", bufs=2) as accp,
            tc.tile_pool(name="work", bufs=2) as workp,
            tc.tile_pool(name="sp", bufs=3) as sp,
            tc.tile_pool(name="tailp", bufs=1, space="PSUM") as tailp,
            tc.tile_pool(name="scp", bufs=2, space="PSUM") as scp,
            tc.tile_pool(name="otp", bufs=2, space="PSUM") as otp,
        ):
            m01_sb = constp.tile([128, 256], BF16, tag="m01")
            nc.sync.dma_start(m01_sb[:], m01d[:])

            # ---- input DMAs, pair-interleaved so both pairs start fast ----
            qkt_sbs, sg_sbs, vvkn_sbs, accs, tacc = [], [], [], [], []
            for p in range(P):
                qkt_sb = iop.tile([65, 2, L], BF16, tag=f"qkt{p}")
                sg_sb = iop.tile([64, 2, L], BF16, tag=f"sg{p}")
                vvkn_sb = iop.tile([128, NB * 65 + NB * 64], BF16,
                                   tag=f"vvkn{p}")
                qkt_sbs.append(qkt_sb)
                sg_sbs.append(sg_sb)
                vvkn_sbs.append(vvkn_sb)
                a = accp.tile([65, NB, 384], BF16, tag=f"acc{p}")
                t = accp.tile([128, 2 * NB], F32, tag=f"tails{p}")
                nc.gpsimd.memset(t[:], 0.0)
                accs.append(a); tacc.append(t)
            for p in range(P):
                nc.sync.dma_start(qkt_sbs[p][:, :, 0:1024], qkt[p, :, :, 0:1024])
                nc.sync.dma_start(qkt_sbs[p][:, :, 1024:2048],
                                  qkt[p, :, :, 1024:2048])
            for p in range(P):
                nc.sync.dma_start(sg_sbs[p][:, :, 0:1024], sg[p, :, :, 0:1024])
            for p in range(P):
                nc.sync.dma_start(vvkn_sbs[p][:], vvkn[p])
            for p in range(P):
                nc.sync.dma_start(sg_sbs[p][:, :, 1024:2048],
                                  sg[p, :, :, 1024:2048])

            def vv_ap(p, i):
                return vvkn_sbs[p][:, i * 65:(i + 1) * 65]

            def sgkn_ap(p, c):
                return vvkn_sbs[p][:, NB * 65 + c * 64:NB * 65 + (c + 1) * 64]

            # per-pair persistent [128, 1024] tail psum tiles (2 banks each)
            tp0 = tailp.tile([128, 1024], F32, tag="tp0")
            tp1 = tailp.tile([128, 1024], F32, tag="tp1")
            tps = [tp0, tp1]

            def tail_pass(p, i, lo, hi, acccol):
                """scores q-block i vs s in [lo, hi) -> exp+accum into col."""
                tp_ = tps[p]
                q0 = i * 128
                qt_sb = qkt_sbs[p][:, 1, :]
                kt_sb = qkt_sbs[p][:, 0, :]
                for off in range(lo, hi, 512):
                    n_ = min(512, hi - off)
                    nc.tensor.matmul(tp_[:, off - lo:off - lo + n_],
                                     qt_sb[:, q0:q0 + 128],
                                     kt_sb[:, off:off + n_],
                                     start=True, stop=True)
                scr = workp.tile([128, 1024], BF16, tag=f"scrap{p}")
                nc.scalar.activation(scr[:, 0:hi - lo], tp_[:, 0:hi - lo],
                                     AF.Exp, scale=TEMP,
                                     accum_out=tacc[p][:, acccol:acccol + 1])

            def tail_block(p, i):
                w = i * 128
                tail_pass(p, i, 0, min(w, 1024), i)
                if w > 1024:
                    tail_pass(p, i, 1024, w, NB + i)

            # hoisted: block-15 tails for both pairs, pass-interleaved
            for p in range(P):
                tail_pass(p, NB - 1, 0, 1024, NB - 1)
            for p in range(P):
                tail_pass(p, NB - 1, 1024, (NB - 1) * 128, 2 * NB - 1)

            s_cur = [None, None]
            prev = [None, None]
            for n in range(NB):
                for p in range(P):
                    qw = 256 if n < NB - 1 else 128
                    c0, c1 = n * 128, (n + 1) * 128
                    kt_sb = qkt_sbs[p][:, 0, :]
                    qt_sb = qkt_sbs[p][:, 1, :]
                    sgk_sb = sg_sbs[p][:, 0, :]
                    sgq_sb = sg_sbs[p][:, 1, :]

                    sc = scp.tile([128, 512], F32, tag="sc")
                    nc.tensor.matmul(sc[:, 0:qw], kt_sb[:, c0:c1],
                                     qt_sb[:, c0:c0 + qw],
                                     start=True, stop=True)
                    nc.tensor.matmul(sc[:, 256:384], sgk_sb[:, c0:c1],
                                     sgq_sb[:, c0:c1], start=True, stop=True)
                    if prev[p] is not None:
                        pot = prev[p]["ot"]
                        pn = n - 1
                        nc.tensor.matmul(pot[:], vv_ap(p, pn),
                                         prev[p]["stat"][:], start=True,
                                         stop=(pn == 0), skip_group_check=True)
                        if pn > 0:
                            nc.tensor.matmul(pot[:, 256:384],
                                             prev[p]["s_before"][:],
                                             sgq_sb[:, pn * 128:pn * 128 + 128],
                                             start=False, stop=True,
                                             skip_group_check=True)
                        nc.vector.tensor_copy(accs[p][:, pn, :], pot[:])
                    nc.tensor.matmul(sc[0:64, 384:449], sgkn_ap(p, n),
                                     vv_ap(p, n), start=True, stop=True)

                    st_e = workp.tile([128, 256], BF16, tag="st_e")
                    nc.scalar.activation(st_e[:, 0:qw], sc[:, 0:qw], AF.Exp,
                                         scale=TEMP)
                    if 1 <= n < NB - 1:
                        tail_block(p, n)

                    stat = workp.tile([128, 384], BF16, tag="stat")
                    nc.vector.tensor_tensor(stat[:, 256:384], sc[:, 256:384],
                                            m01_sb[:, 0:128], OP.mult)
                    nc.vector.tensor_tensor(stat[:, 0:qw], st_e[:, 0:qw],
                                            m01_sb[:, 0:qw], OP.mult)
                    if qw < 256:
                        nc.vector.memset(stat[:, 128:256], 0.0)
                    s_before = s_cur[p]
                    s_nxt = sp.tile([64, 65], BF16, tag=f"s{p}")
                    if n == 0:
                        nc.vector.tensor_copy(s_nxt[:], sc[0:64, 384:449])
                    else:
                        nc.vector.scalar_tensor_tensor(s_nxt[:], s_cur[p][:], 1.0,
                                                       sc[0:64, 384:449],
                                                       OP.mult, OP.add)
                    s_cur[p] = s_nxt

                    ot = otp.tile([65, 384], F32, tag="ot")
                    prev[p] = {"ot": ot, "stat": stat, "s_before": s_before}
                    if n in (5, 9, 13):
                        nc.sync.dma_start(svlv[p, :, n - 5:n - 1, :],
                                          accs[p][:, n - 5:n - 1, :])

            # ---- epilogue: finish last block for both pairs ----
            for p in range(P):
                sgq_sb = sg_sbs[p][:, 1, :]
                pot = prev[p]["ot"]
                pn = NB - 1
                nc.tensor.matmul(pot[:], vv_ap(p, pn), prev[p]["stat"][:],
                                 start=True, stop=False, skip_group_check=True)
                nc.tensor.matmul(pot[:, 256:384], prev[p]["s_before"][:],
                                 sgq_sb[:, pn * 128:pn * 128 + 128],
                                 start=False, stop=True, skip_group_check=True)
                nc.vector.tensor_copy(accs[p][:, pn, :], pot[:])
                nc.scalar.dma_start(svlv[p, :, 12:NB, :], accs[p][:, 12:NB, :])
                nc.scalar.dma_start(tails[p], tacc[p][:])

    nc.compile()
    return nc


def host_prep(queries, keys, values, key_lengths_mask, blend):
    """Build per-core in_maps from full inputs."""
    q = np.ascontiguousarray(np.transpose(queries, (0, 2, 1, 3)))  # [N,H,L,E]
    k = np.ascontiguousarray(np.transpose(keys, (0, 2, 1, 3)))
    v = np.ascontiguousarray(np.transpose(values, (0, 2, 1, 3)))
    q = q.reshape(N * H, L, E).astype(np.float32)
    k = k.reshape(N * H, L, E).astype(np.float32)
    v = v.reshape(N * H, L, E).astype(np.float32)
    klm = np.asarray(key_lengths_mask, np.float32)  # [N, L]

    ii = np.arange(128)[:, None]
    cc = np.arange(256)[None, :]
    m01 = ((cc - ii >= 0) & (cc - ii <= 128)).astype(np.float32)

    in_maps = []
    for core in range(NCORES):
        qkts, sgs, vvkns = [], [], []
        for p in range(PAIRS_PER_CORE):
            g = core * PAIRS_PER_CORE + p
            n = g // H
            qg, kg, vg = q[g], k[g], v[g]          # [L, E]
            kl = klm[n]                             # [L]
            i01 = (kl > 0).astype(np.float32)

            qkt_p = np.empty((65, 2, L), np.float32)
            qkt_p[0:64, 0] = kg.T
            qkt_p[64, 0] = -1e9 * (1.0 - i01)
            qkt_p[0:64, 1] = qg.T
            qkt_p[64, 1] = 1.0

            phiq = np.tanh(qg) + 1.0
            phik = np.tanh(kg) + 1.0
            sg_p = np.empty((64, 2, L), np.float32)
            sg_p[:, 0] = phik.T
            sg_p[:, 1] = phiq.T

            vv_full = np.empty((L, 65), np.float32)
            vv_full[:, 0:64] = vg * kl[:, None]
            vv_full[:, 64] = kl
            vv_p = vv_full.reshape(NB, 128, 65).transpose(1, 0, 2)
            sgkn_p = phik.reshape(NB, 128, 64).transpose(1, 0, 2).reshape(128, NB * 64)
            vvkn_p = np.concatenate([vv_p.reshape(128, NB * 65), sgkn_p], axis=1)

            qkts.append(qkt_p.astype(ml_dtypes.bfloat16))
            sgs.append(sg_p.astype(ml_dtypes.bfloat16))
            vvkns.append(vvkn_p.astype(ml_dtypes.bfloat16))

        in_maps.append({
            "qkt": np.ascontiguousarray(np.stack(qkts)),
            "sg": np.ascontiguousarray(np.stack(sgs)),
            "vvkn": np.ascontiguousarray(np.stack(vvkns)),
            "m01d": np.ascontiguousarray(m01.astype(ml_dtypes.bfloat16)),
        })
    return in_maps


def assemble(results, blend):
    """Normalize + blend on host from raw numerators/denominators."""
    b = float(np.asarray(blend).reshape(-1)[0])
    full = np.empty((N, H, L, E), np.float32)
    for core in range(NCORES):
        r = results[core]
        svlv = np.asarray(r["svlv"], dtype=np.float32)   # [P, 65, NB, 384]
        tails = np.asarray(r["tails"])                   # [P, 128, 2*NB]
        for p in range(PAIRS_PER_CORE):
            g = core * PAIRS_PER_CORE + p
            n, h = g // H, g % H
            sv = svlv[p, :, :, 0:256]       # [65, block, 256]
            lv = svlv[p, :, :, 256:384]     # [65, chunk, 128]
            tl_sum = tails[p, :, 0:NB] + tails[p, :, NB:2 * NB]
            den = tl_sum.T + sv[64, :, 0:128]            # [NB, 128]
            num = sv[0:64, :, 0:128].copy()              # [64, NB, 128]
            num[:, 1:, :] += sv[0:64, 0:NB - 1, 128:256]
            lvn = lv[0:64]                               # [64, NB, 128]
            lvd = lv[64]                                 # [NB, 128]
            o = (b * num / den[None] +
                 (1.0 - b) * lvn / (lvd[None] + EPS))    # [64, NB, 128]
            full[n, h] = o.transpose(1, 2, 0).reshape(L, E)
    return np.ascontiguousarray(np.transpose(full, (0, 2, 1, 3)))


def kernel(queries, keys, values, key_lengths_mask, blend, _trace=False):
    if "nc" not in _cached:
        _cached["nc"] = build_nc()
    nc = _cached["nc"]
    in_maps = host_prep(queries, keys, values, key_lengths_mask, blend)
    res = bass_utils.run_bass_kernel_spmd(nc, in_maps, core_ids=list(range(NCORES)),
                                          trace=_trace)
    _cached["last_results"] = res
    return assemble(res.results, blend)
